# revision 4
# baseline (speedup 1.0000x reference)
"""DepthToSpace (block_size=2, CRD layout) Trainium2 Bass kernel.

x: [16, 256, 128, 128] f32  ->  out: [16, 64, 256, 256] f32
out[b, dd, 2h+i, 2w+k] = x[b, (2i+k)*64 + dd, h, w]

Sharding: batch dim split across 8 NeuronCores (2 examples per core),
no communication. Per core the kernel is a pure reshuffle:
  - partition axis p = (b_local, dd) = 2*64 = 128 partitions
  - per h-tile: one 4 MiB HBM->SBUF DMA (contiguous 8 KiB runs),
    DVE strided copies do the 2x2 pixel-shuffle interleave in SBUF,
    one 4 MiB SBUF->HBM DMA (contiguous 32 KiB runs).
"""

import numpy as np

import concourse.bass as bass  # noqa: F401  (registers AP machinery)
import concourse.tile as tile
from concourse import bacc, bass_utils, mybir

# Problem shape (hardcoded per spec).
B, C, H, W = 16, 256, 128, 128
NCORES = 8
BL = B // NCORES  # local batch per core = 2
D = C // 4        # out channels = 64
HT = 16           # input rows per tile
NT = H // HT      # tiles per core = 8

_cached_nc = None


def _build(reps: int = 1):
    nc = bacc.Bacc(
        "TRN2",
        target_bir_lowering=False,
        debug=False,
        num_devices=NCORES,
    )
    x = nc.dram_tensor(
        "x", [BL, C, H, W], mybir.dt.float32, kind="ExternalInput"
    ).ap()
    out = nc.dram_tensor(
        "out", [BL, D, 2 * H, 2 * W], mybir.dt.float32, kind="ExternalOutput"
    ).ap()

    # x viewed as [b, dd, cb, h, w] where channel c = cb*64 + dd, cb = 2i+k.
    xr = x.rearrange("b (cb dd) h w -> b dd cb h w", cb=4)

    def body(nc, tc, inp, outp):
        for t in range(NT):
            h0 = t * HT
            it = inp.tile([128, 4 * HT * W], mybir.dt.float32)
            # SBUF layout: it[p=(b,dd), cb*HT*W + h*W + w]
            # DMA APs allow at most 3 dims; (b, dd, cb, run) is 4, so
            # split per local batch (64 partitions each) across the two
            # HWDGE rings so the halves overlap.
            nc.sync.dma_start(it[0:64, :], xr[0, :, :, h0 : h0 + HT, :])
            nc.scalar.dma_start(it[64:128, :], xr[1, :, :, h0 : h0 + HT, :])

            ot = outp.tile([128, HT * 4 * W], mybir.dt.float32)
            # ot[p, h*(2*2W) + i*(2W) + w*2 + k] = it[p, (2i+k)*HT*W + h*W + w]
            sv = it[:].rearrange("p (i k h w) -> p i h w k", i=2, k=2, h=HT, w=W)
            dv = ot[:].rearrange("p (h i w k) -> p i h w k", h=HT, i=2, w=W, k=2)
            for i in range(2):
                nc.vector.tensor_copy(dv[:, i], sv[:, i])

            # Output split per local batch across the two HWDGE rings.
            nc.scalar.dma_start(
                out[0, :, 2 * h0 : 2 * h0 + 2 * HT, :], ot[0:64, :]
            )
            nc.sync.dma_start(
                out[1, :, 2 * h0 : 2 * h0 + 2 * HT, :], ot[64:128, :]
            )

    with tile.TileContext(nc) as tc:
        with tc.tile_pool(name="inp", bufs=2) as inp, tc.tile_pool(
            name="outp", bufs=2
        ) as outp:
            if reps == 1:
                body(nc, tc, inp, outp)
            else:
                with tc.For_i(0, reps, 1):
                    body(nc, tc, inp, outp)
    nc.compile()
    return nc


def kernel(x: np.ndarray) -> np.ndarray:
    global _cached_nc
    if _cached_nc is None:
        _cached_nc = _build()
    nc = _cached_nc

    x = np.ascontiguousarray(x, dtype=np.float32)
    in_maps = [
        {"x": np.ascontiguousarray(x[c * BL : (c + 1) * BL])} for c in range(NCORES)
    ]
    res = bass_utils.run_bass_kernel_spmd(nc, in_maps, core_ids=list(range(NCORES)))
    return np.concatenate([r["out"] for r in res.results], axis=0)
